# revision 5
# baseline (speedup 1.0000x reference)
"""Binarized-MLP (784->768->768->768->10, BN+hardtanh, log_softmax) on 8 trn2 cores.

Strategy: pure data-parallel over the batch (16384 rows -> 8 x 2048), weights
replicated.  Activations live feature-major [H, B] in SBUF so no transposes
are needed between layers; x is transposed on host.

Precision:
- fc1: x is split into fp16 hi + fp16 lo*2^12 halves stacked along K
  (K=1568) with weights [sign(W1); sign(W1)*2^-12]; products are exact
  (weights are powers of two), PSUM accumulates in fp32 -> ~fp32-exact.
  This matters: the downstream sign() layers chaotically amplify fc1 error.
- fc2/fc3: +-1 x +-1 products accumulated in fp32 PSUM: bit-exact in bf16.
- fc4: h3 kept as fp16 hi + fp16 lo (unscaled residual; error floor ~6e-8)
  against fp16(W4) -> logit error ~1e-4 absolute, well inside tolerance.
- BN (eval) + layer bias folded host-side into per-feature scale/bias;
  fc1/fc2 epilogue is a single scalar-engine Sign activation per tile.
"""

import numpy as np
import ml_dtypes

import bass_rust
import concourse.bass as bass
import concourse.tile as tile
from concourse import mybir
from concourse.bass_utils import run_bass_kernel_spmd

EPS = 1e-5
NCORES = 8
B, D, H, C = 16384, 784, 768, 10
BC = B // NCORES          # 2048 rows per core
BLK = 512                 # batch columns per block
NBLK = BC // BLK          # 4
K1 = 2 * D                # 1568 stacked hi/lo contraction for fc1
K1C = (K1 + 127) // 128   # 13 chunks (last has 32 rows)
HT = H // 128             # 6 feature tiles

F16 = mybir.dt.float16
BF16 = mybir.dt.bfloat16
F32 = mybir.dt.float32

_cache = {}


def _split_waits(nc, max_waits=1):
    """Hoist excess sync waits into standalone InstEventSemaphore ops.

    The walrus build in this environment rejects instructions carrying more
    than one sync-wait command ("Too many sync wait commands"), but Tile
    freely emits several.  Splitting them into preceding same-engine event
    waits is semantically identical (the sequencer stalls either way).
    """
    n_new = 0
    for f in nc.m.functions:
        for b in f.blocks:
            il = b.instructions
            i = 0
            while i < len(il):
                inst = il[i]
                si = inst.sync_info
                if si and si.on_wait and len(si.on_wait) > max_waits:
                    waits = list(si.on_wait)
                    keep, extra = waits[-max_waits:], waits[:-max_waits]
                    evs = []
                    for w in extra:
                        ev = bass_rust.InstEventSemaphore(
                            name=f"EVW-{n_new}", engine=inst.engine,
                            ins=[], outs=[])
                        ev.sync_info = mybir.SyncInfo(on_wait=[w], on_update=[])
                        evs.append(ev)
                        n_new += 1
                    inst.sync_info = mybir.SyncInfo(
                        on_wait=keep, on_update=list(si.on_update or []))
                    il[i:i] = evs
                    i += len(evs)
                i += 1
    return n_new


def _build():
    nc = bass.Bass()

    xbig_d = nc.dram_tensor("xbig", [K1, BC], F16, kind="ExternalInput")
    w1_d = nc.dram_tensor("w1", [K1, H], F16, kind="ExternalInput")
    w2_d = nc.dram_tensor("w2", [H, H], BF16, kind="ExternalInput")
    w3_d = nc.dram_tensor("w3", [H, H], BF16, kind="ExternalInput")
    w4_d = nc.dram_tensor("w4", [H, C], F16, kind="ExternalInput")
    par_d = nc.dram_tensor("par", [128, 36], F32, kind="ExternalInput")
    b4_d = nc.dram_tensor("b4b", [128, C], F32, kind="ExternalInput")
    out_d = nc.dram_tensor("out", [BC, C], F32, kind="ExternalOutput")

    with tile.TileContext(nc) as tc:
        with (
            tc.tile_pool(name="wpool", bufs=1) as wpool,
            tc.tile_pool(name="xpool", bufs=2) as xpool,
            tc.tile_pool(name="hpool", bufs=2) as hpool,
            tc.tile_pool(name="tmppool", bufs=3) as tmppool,
            tc.tile_pool(name="spool", bufs=5) as spool,
            tc.tile_pool(name="opool", bufs=4) as opool,
            tc.tile_pool(name="pspool", bufs=4, space="PSUM") as pspool,
            tc.tile_pool(name="ps4pool", bufs=2, space="PSUM") as ps4pool,
        ):
            # ---- static loads (weights + folded BN params) ----
            w1sb = wpool.tile([128, K1C, H], F16)
            for c in range(K1C):
                rc = min(128, K1 - c * 128)
                nc.sync.dma_start(
                    out=w1sb[:rc, c, :], in_=w1_d[c * 128 : c * 128 + rc, :]
                )
            w2sb = wpool.tile([128, HT, H], BF16)
            w3sb = wpool.tile([128, HT, H], BF16)
            for c in range(HT):
                nc.sync.dma_start(
                    out=w2sb[:, c, :], in_=w2_d[c * 128 : (c + 1) * 128, :]
                )
                nc.sync.dma_start(
                    out=w3sb[:, c, :], in_=w3_d[c * 128 : (c + 1) * 128, :]
                )
            w4sb = wpool.tile([128, HT, C], F16)
            for c in range(HT):
                nc.sync.dma_start(
                    out=w4sb[:, c, :], in_=w4_d[c * 128 : (c + 1) * 128, :]
                )
            psb = wpool.tile([128, 36], F32)
            nc.sync.dma_start(out=psb[:, :], in_=par_d[:, :])
            b4sb = wpool.tile([128, C], F32)
            nc.sync.dma_start(out=b4sb[:, :], in_=b4_d[:, :])

            def pcol(p, m):  # param column: p in {s1,t1,s2,t2,s3,t3}
                i = p * 6 + m
                return psb[:, i : i + 1]

            # ---- main loop over batch blocks ----
            for blk in range(NBLK):
                b0 = blk * BLK
                xsb = xpool.tile([128, K1C, BLK], F16)
                for c in range(K1C):
                    rc = min(128, K1 - c * 128)
                    nc.sync.dma_start(
                        out=xsb[:rc, c, :],
                        in_=xbig_d[c * 128 : c * 128 + rc, b0 : b0 + BLK],
                    )

                # fc1 + BN1 + sign  -> h1 in {+-1} bf16, feature-major
                h1 = hpool.tile([128, HT, BLK], BF16, tag="h1")
                for m in range(HT):
                    ps = pspool.tile([128, BLK], F32)
                    for c in range(K1C):
                        rc = min(128, K1 - c * 128)
                        nc.tensor.matmul(
                            ps[:, :],
                            lhsT=w1sb[:rc, c, m * 128 : (m + 1) * 128],
                            rhs=xsb[:rc, c, :],
                            start=(c == 0),
                            stop=(c == K1C - 1),
                        )
                    nc.scalar.activation(
                        out=h1[:, m, :],
                        in_=ps[:, :],
                        func=mybir.ActivationFunctionType.Sign,
                        scale=pcol(0, m),
                        bias=pcol(1, m),
                    )

                # fc2 + BN2 + sign
                h2 = hpool.tile([128, HT, BLK], BF16, tag="h2")
                for m in range(HT):
                    ps = pspool.tile([128, BLK], F32)
                    for c in range(HT):
                        nc.tensor.matmul(
                            ps[:, :],
                            lhsT=w2sb[:, c, m * 128 : (m + 1) * 128],
                            rhs=h1[:, c, :],
                            start=(c == 0),
                            stop=(c == HT - 1),
                        )
                    nc.scalar.activation(
                        out=h2[:, m, :],
                        in_=ps[:, :],
                        func=mybir.ActivationFunctionType.Sign,
                        scale=pcol(2, m),
                        bias=pcol(3, m),
                    )

                # fc3 + BN3 + hardtanh -> h3 as fp16 hi (tiles 0-5) +
                # fp16 lo residual (tiles 6-11)
                h3 = hpool.tile([128, 2 * HT, BLK], F16, tag="h3")
                for m in range(HT):
                    ps = pspool.tile([128, BLK], F32)
                    for c in range(HT):
                        nc.tensor.matmul(
                            ps[:, :],
                            lhsT=w3sb[:, c, m * 128 : (m + 1) * 128],
                            rhs=h2[:, c, :],
                            start=(c == 0),
                            stop=(c == HT - 1),
                        )
                    zt = tmppool.tile([128, BLK], F32, tag="z3")
                    nc.scalar.activation(
                        out=zt[:, :],
                        in_=ps[:, :],
                        func=mybir.ActivationFunctionType.Identity,
                        scale=pcol(4, m),
                        bias=pcol(5, m),
                    )
                    ct = tmppool.tile([128, BLK], F32, tag="c3")
                    nc.vector.tensor_scalar(
                        out=ct[:, :],
                        in0=zt[:, :],
                        scalar1=1.0,
                        scalar2=-1.0,
                        op0=mybir.AluOpType.min,
                        op1=mybir.AluOpType.max,
                    )
                    nc.vector.tensor_copy(out=h3[:, m, :], in_=ct[:, :])
                    nc.vector.tensor_sub(h3[:, HT + m, :], ct[:, :], h3[:, m, :])

                # fc4 + log_softmax, batch-major [128, 10] per tile.
                # Phase A (per tile): matmul, +b4, -max, exp+rowsum.
                zs, nmxs, ses = [], [], []
                for t in range(BLK // 128):
                    ps4 = ps4pool.tile([128, C], F32)
                    for c in range(2 * HT):
                        nc.tensor.matmul(
                            ps4[:, :],
                            lhsT=h3[:, c, t * 128 : (t + 1) * 128],
                            rhs=w4sb[:, c % HT, :],
                            start=(c == 0),
                            stop=(c == 2 * HT - 1),
                        )
                    z = spool.tile([128, C], F32, tag=f"z{t}")
                    nc.vector.tensor_add(z[:, :], ps4[:, :], b4sb[:, :])
                    nmx = spool.tile([128, 1], F32, tag=f"nmx{t}")
                    nc.vector.tensor_reduce(
                        out=nmx[:, :],
                        in_=z[:, :],
                        axis=mybir.AxisListType.X,
                        op=mybir.AluOpType.max,
                        negate=True,
                    )
                    e = spool.tile([128, C], F32, tag=f"e{t}")
                    se = spool.tile([128, 1], F32, tag=f"se{t}")
                    nc.scalar.activation(
                        out=e[:, :],
                        in_=z[:, :],
                        func=mybir.ActivationFunctionType.Exp,
                        bias=nmx[:, :],
                        scale=1.0,
                        accum_out=se[:, :],
                    )
                    zs.append(z)
                    nmxs.append(nmx)
                    ses.append(se)
                # Phase B (batched Ln, then final subtract + store)
                for t in range(BLK // 128):
                    lse = spool.tile([128, 1], F32, tag=f"lse{t}")
                    nc.scalar.activation(
                        out=lse[:, :],
                        in_=ses[t][:, :],
                        func=mybir.ActivationFunctionType.Ln,
                    )
                    off = spool.tile([128, 1], F32, tag=f"off{t}")
                    nc.vector.tensor_sub(off[:, :], lse[:, :], nmxs[t][:, :])
                    ot = opool.tile([128, C], F32, tag="ot")
                    nc.vector.tensor_scalar(
                        out=ot[:, :],
                        in0=zs[t][:, :],
                        scalar1=off[:, 0:1],
                        scalar2=None,
                        op0=mybir.AluOpType.subtract,
                    )
                    nc.sync.dma_start(
                        out=out_d[b0 + t * 128 : b0 + (t + 1) * 128, :],
                        in_=ot[:, :],
                    )
    _split_waits(nc)
    return nc


def _prep(inputs):
    """Host-side constant folding + sharding. Returns per-core in_maps."""
    f32 = np.float32
    x = np.asarray(inputs["x"], f32)
    W1 = np.asarray(inputs["W1"], f32)
    W2 = np.asarray(inputs["W2"], f32)
    W3 = np.asarray(inputs["W3"], f32)
    W4 = np.asarray(inputs["W4"], f32)
    b1 = np.asarray(inputs["b1"], f32)
    b2 = np.asarray(inputs["b2"], f32)
    b3 = np.asarray(inputs["b3"], f32)
    b4 = np.asarray(inputs["b4"], f32)

    def fold(g, be, m, v, b):
        s = (g / np.sqrt(v + EPS)).astype(f32)
        t = (b * s + be - m * s).astype(f32)
        return s, t

    s1, t1 = fold(np.asarray(inputs["g1"], f32), np.asarray(inputs["be1"], f32),
                  np.asarray(inputs["m1"], f32), np.asarray(inputs["v1"], f32), b1)
    s2, t2 = fold(np.asarray(inputs["g2"], f32), np.asarray(inputs["be2"], f32),
                  np.asarray(inputs["m2"], f32), np.asarray(inputs["v2"], f32), b2)
    s3, t3 = fold(np.asarray(inputs["g3"], f32), np.asarray(inputs["be3"], f32),
                  np.asarray(inputs["m3"], f32), np.asarray(inputs["v3"], f32), b3)

    def sgn(w):
        return np.where(w >= 0, f32(1.0), f32(-1.0))

    w1sT = sgn(W1).T                                   # [784, 768]
    w1big = np.ascontiguousarray(
        np.concatenate([w1sT, w1sT * f32(2.0 ** -12)], 0)
    ).astype(np.float16)                               # [1568, 768]
    w2big = np.ascontiguousarray(sgn(W2).T).astype(ml_dtypes.bfloat16)
    w3big = np.ascontiguousarray(sgn(W3).T).astype(ml_dtypes.bfloat16)
    w4big = np.ascontiguousarray(W4.T).astype(np.float16)   # [768, 10]

    par = np.ascontiguousarray(np.concatenate(
        [s1.reshape(6, 128).T, t1.reshape(6, 128).T,
         s2.reshape(6, 128).T, t2.reshape(6, 128).T,
         s3.reshape(6, 128).T, t3.reshape(6, 128).T], axis=1
    )).astype(f32)                                     # [128, 36]
    b4b = np.ascontiguousarray(np.broadcast_to(b4, (128, C))).astype(f32)

    xT = np.ascontiguousarray(x.T)                     # [784, 16384]
    xhi = xT.astype(np.float16)
    xlo = ((xT - xhi.astype(f32)) * f32(4096.0)).astype(np.float16)
    xbig = np.concatenate([xhi, xlo], 0)               # [1568, 16384]

    in_maps = []
    for i in range(NCORES):
        sl = np.ascontiguousarray(xbig[:, i * BC : (i + 1) * BC])
        in_maps.append({
            "xbig": sl, "w1": w1big, "w2": w2big, "w3": w3big,
            "w4": w4big, "par": par, "b4b": b4b,
        })
    return in_maps


TRACE = False
LAST_RESULT = None


def kernel(**inputs):
    global LAST_RESULT
    if "nc" not in _cache:
        _cache["nc"] = _build()
    nc = _cache["nc"]
    in_maps = _prep(inputs)
    res = run_bass_kernel_spmd(nc, in_maps, list(range(NCORES)), trace=TRACE)
    LAST_RESULT = res
    out = np.concatenate([np.asarray(r["out"]) for r in res.results], axis=0)
    return out.astype(np.float32)


# revision 6
# speedup vs baseline: 403.6573x; 403.6573x over previous
"""Binarized-MLP (784->768->768->768->10, BN+hardtanh, log_softmax) on 8 trn2 cores.

Strategy: pure data-parallel over the batch (16384 rows -> 8 x 2048), weights
replicated.  Activations live feature-major [H, B] in SBUF so no transposes
are needed between layers; x is transposed on host.

Precision:
- fc1: x is split into fp16 hi + fp16 lo*2^12 halves stacked along K
  (K=1568) with weights [sign(W1); sign(W1)*2^-12]; products are exact
  (weights are powers of two), PSUM accumulates in fp32 -> ~fp32-exact.
  This matters: the downstream sign() layers chaotically amplify fc1 error.
- fc2/fc3: +-1 x +-1 products accumulated in fp32 PSUM are exact in ANY
  float dtype -> fp8e4 with perf_mode=DoubleRow (2 K-tiles per pass).
- fc4: h3 kept as fp16 hi + fp16 lo (unscaled residual) against fp16(W4).
- BN (eval) + layer bias folded host-side into per-feature scale/bias;
  fc1/fc2 epilogue is a single scalar-engine Sign activation per tile.
"""

import numpy as np
import ml_dtypes

import bass_rust
import concourse.bass as bass
import concourse.tile as tile
from concourse import mybir
from concourse.bass_utils import run_bass_kernel_spmd

EPS = 1e-5
NCORES = 8
B, D, H, C = 16384, 784, 768, 10
BC = B // NCORES          # 2048 rows per core
BLK = 512                 # batch columns per block
NBLK = BC // BLK          # 4
K1 = 2 * D                # 1568 stacked hi/lo contraction for fc1
K1C = (K1 + 127) // 128   # 13 chunks (last has 32 rows)
HT = H // 128             # 6 feature tiles
HP = HT // 2              # 3 DoubleRow k-pairs

F16 = mybir.dt.float16
BF16 = mybir.dt.bfloat16
F8 = mybir.dt.float8e4
F32 = mybir.dt.float32
NP_F8 = mybir.dt.np(mybir.dt.float8e4)

_cache = {}


def _split_waits(nc, max_waits=1):
    """Hoist excess sync waits into standalone InstEventSemaphore ops.

    The walrus build in this environment rejects instructions carrying more
    than one sync-wait command ("Too many sync wait commands"), but Tile
    freely emits several.  Splitting them into preceding same-engine event
    waits is semantically identical (the sequencer stalls either way).
    """
    n_new = 0
    for f in nc.m.functions:
        for b in f.blocks:
            il = b.instructions
            i = 0
            while i < len(il):
                inst = il[i]
                si = inst.sync_info
                if si and si.on_wait and len(si.on_wait) > max_waits:
                    waits = list(si.on_wait)
                    keep, extra = waits[-max_waits:], waits[:-max_waits]
                    evs = []
                    for w in extra:
                        ev = bass_rust.InstEventSemaphore(
                            name=f"EVW-{n_new}", engine=inst.engine,
                            ins=[], outs=[])
                        ev.sync_info = mybir.SyncInfo(on_wait=[w], on_update=[])
                        evs.append(ev)
                        n_new += 1
                    inst.sync_info = mybir.SyncInfo(
                        on_wait=keep, on_update=list(si.on_update or []))
                    il[i:i] = evs
                    i += len(evs)
                i += 1
    return n_new


def _build(repeat=1):
    nc = bass.Bass()

    xbig_d = nc.dram_tensor("xbig", [K1, BC], F16, kind="ExternalInput")
    w1_d = nc.dram_tensor("w1", [K1, H], F16, kind="ExternalInput")
    w2_d = nc.dram_tensor("w2", [128, HP * 2 * H], F8, kind="ExternalInput")
    w3_d = nc.dram_tensor("w3", [128, HP * 2 * H], F8, kind="ExternalInput")
    w4_d = nc.dram_tensor("w4", [H, C], F16, kind="ExternalInput")
    par_d = nc.dram_tensor("par", [128, 36], F32, kind="ExternalInput")
    b4_d = nc.dram_tensor("b4b", [128, C], F32, kind="ExternalInput")
    out_d = nc.dram_tensor("out", [BC, C], F32, kind="ExternalOutput")

    with tile.TileContext(nc) as tc:
        with (
            tc.tile_pool(name="wpool", bufs=1) as wpool,
            tc.tile_pool(name="xpool", bufs=2) as xpool,
            tc.tile_pool(name="hpool", bufs=2) as hpool,
            tc.tile_pool(name="tmppool", bufs=3) as tmppool,
            tc.tile_pool(name="spool", bufs=5) as spool,
            tc.tile_pool(name="opool", bufs=4) as opool,
            tc.tile_pool(name="pspool", bufs=4, space="PSUM") as pspool,
            tc.tile_pool(name="ps4pool", bufs=2, space="PSUM") as ps4pool,
        ):
            # ---- static loads, ordered so block-0 compute starts early:
            # w1 + params first, then x block 0, then the later-layer weights.
            w1sb = wpool.tile([128, K1C, H], F16)
            for c in range(K1C):
                rc = min(128, K1 - c * 128)
                nc.sync.dma_start(
                    out=w1sb[:rc, c, :], in_=w1_d[c * 128 : c * 128 + rc, :]
                )
            psb = wpool.tile([128, 36], F32)
            nc.sync.dma_start(out=psb[:, :], in_=par_d[:, :])
            b4sb = wpool.tile([128, C], F32)
            nc.sync.dma_start(out=b4sb[:, :], in_=b4_d[:, :])

            def load_x(blk):
                xsb = xpool.tile([128, K1C, BLK], F16)
                b0 = blk * BLK
                for c in range(K1C):
                    rc = min(128, K1 - c * 128)
                    nc.sync.dma_start(
                        out=xsb[:rc, c, :],
                        in_=xbig_d[c * 128 : c * 128 + rc, b0 : b0 + BLK],
                    )
                return xsb

            xsb0 = load_x(0)

            w2sb = wpool.tile([128, HP, 2, H], F8)
            nc.sync.dma_start(out=w2sb[:, :, :, :], in_=w2_d[:, :])
            w3sb = wpool.tile([128, HP, 2, H], F8)
            nc.sync.dma_start(out=w3sb[:, :, :, :], in_=w3_d[:, :])
            w4sb = wpool.tile([128, HT, C], F16)
            for c in range(HT):
                nc.sync.dma_start(
                    out=w4sb[:, c, :], in_=w4_d[c * 128 : (c + 1) * 128, :]
                )

            def pcol(p, m):  # param column: p in {s1,t1,s2,t2,s3,t3}
                i = p * 6 + m
                return psb[:, i : i + 1]

            # ---- main loop over batch blocks ----
            for rep in range(repeat):
                for blk in range(NBLK):
                    b0 = blk * BLK
                    xsb = xsb0 if (rep == 0 and blk == 0) else load_x(blk)

                    # fc1 + BN1 + sign -> h1 {+-1} fp8, DoubleRow-paired
                    h1 = hpool.tile([128, HP, 2, BLK], F8, tag="h1")
                    for m in range(HT):
                        ps = pspool.tile([128, BLK], F32)
                        for c in range(K1C):
                            rc = min(128, K1 - c * 128)
                            nc.tensor.matmul(
                                ps[:, :],
                                lhsT=w1sb[:rc, c, m * 128 : (m + 1) * 128],
                                rhs=xsb[:rc, c, :],
                                start=(c == 0),
                                stop=(c == K1C - 1),
                            )
                        nc.scalar.activation(
                            out=h1[:, m // 2, m % 2, :],
                            in_=ps[:, :],
                            func=mybir.ActivationFunctionType.Sign,
                            scale=pcol(0, m),
                            bias=pcol(1, m),
                        )

                    # fc2 + BN2 + sign (fp8 DoubleRow)
                    h2 = hpool.tile([128, HP, 2, BLK], F8, tag="h2")
                    for m in range(HT):
                        ps = pspool.tile([128, BLK], F32)
                        for t in range(HP):
                            nc.tensor.matmul(
                                ps[:, :],
                                lhsT=w2sb[:, t, :, m * 128 : (m + 1) * 128],
                                rhs=h1[:, t, :, :],
                                start=(t == 0),
                                stop=(t == HP - 1),
                                perf_mode=mybir.MatmulPerfMode.DoubleRow,
                            )
                        nc.scalar.activation(
                            out=h2[:, m // 2, m % 2, :],
                            in_=ps[:, :],
                            func=mybir.ActivationFunctionType.Sign,
                            scale=pcol(2, m),
                            bias=pcol(3, m),
                        )

                    # fc3 + BN3 + hardtanh -> h3 fp16 hi (tiles 0-5) +
                    # fp16 lo residual (tiles 6-11)
                    h3 = hpool.tile([128, 2 * HT, BLK], F16, tag="h3")
                    for m in range(HT):
                        ps = pspool.tile([128, BLK], F32)
                        for t in range(HP):
                            nc.tensor.matmul(
                                ps[:, :],
                                lhsT=w3sb[:, t, :, m * 128 : (m + 1) * 128],
                                rhs=h2[:, t, :, :],
                                start=(t == 0),
                                stop=(t == HP - 1),
                                perf_mode=mybir.MatmulPerfMode.DoubleRow,
                            )
                        zt = tmppool.tile([128, BLK], F32, tag="z3")
                        nc.scalar.activation(
                            out=zt[:, :],
                            in_=ps[:, :],
                            func=mybir.ActivationFunctionType.Identity,
                            scale=pcol(4, m),
                            bias=pcol(5, m),
                        )
                        ct = tmppool.tile([128, BLK], F32, tag="c3")
                        nc.vector.tensor_scalar(
                            out=ct[:, :],
                            in0=zt[:, :],
                            scalar1=1.0,
                            scalar2=-1.0,
                            op0=mybir.AluOpType.min,
                            op1=mybir.AluOpType.max,
                        )
                        nc.vector.tensor_copy(out=h3[:, m, :], in_=ct[:, :])
                        nc.vector.tensor_sub(
                            h3[:, HT + m, :], ct[:, :], h3[:, m, :]
                        )

                    # fc4 + log_softmax, batch-major [128, 10] per tile.
                    zs, nmxs, ses = [], [], []
                    for t in range(BLK // 128):
                        ps4 = ps4pool.tile([128, C], F32)
                        for c in range(2 * HT):
                            nc.tensor.matmul(
                                ps4[:, :],
                                lhsT=h3[:, c, t * 128 : (t + 1) * 128],
                                rhs=w4sb[:, c % HT, :],
                                start=(c == 0),
                                stop=(c == 2 * HT - 1),
                            )
                        z = spool.tile([128, C], F32, tag=f"z{t}")
                        nc.vector.tensor_add(z[:, :], ps4[:, :], b4sb[:, :])
                        nmx = spool.tile([128, 1], F32, tag=f"nmx{t}")
                        nc.vector.tensor_reduce(
                            out=nmx[:, :],
                            in_=z[:, :],
                            axis=mybir.AxisListType.X,
                            op=mybir.AluOpType.max,
                            negate=True,
                        )
                        e = spool.tile([128, C], F32, tag=f"e{t}")
                        se = spool.tile([128, 1], F32, tag=f"se{t}")
                        nc.scalar.activation(
                            out=e[:, :],
                            in_=z[:, :],
                            func=mybir.ActivationFunctionType.Exp,
                            bias=nmx[:, :],
                            scale=1.0,
                            accum_out=se[:, :],
                        )
                        zs.append(z)
                        nmxs.append(nmx)
                        ses.append(se)
                    for t in range(BLK // 128):
                        lse = spool.tile([128, 1], F32, tag=f"lse{t}")
                        nc.scalar.activation(
                            out=lse[:, :],
                            in_=ses[t][:, :],
                            func=mybir.ActivationFunctionType.Ln,
                        )
                        off = spool.tile([128, 1], F32, tag=f"off{t}")
                        nc.vector.tensor_sub(off[:, :], lse[:, :], nmxs[t][:, :])
                        ot = opool.tile([128, C], F32, tag="ot")
                        nc.vector.tensor_scalar(
                            out=ot[:, :],
                            in0=zs[t][:, :],
                            scalar1=off[:, 0:1],
                            scalar2=None,
                            op0=mybir.AluOpType.subtract,
                        )
                        nc.sync.dma_start(
                            out=out_d[b0 + t * 128 : b0 + (t + 1) * 128, :],
                            in_=ot[:, :],
                        )
    _split_waits(nc)
    return nc


def _prep(inputs):
    """Host-side constant folding + sharding. Returns per-core in_maps."""
    f32 = np.float32
    x = np.asarray(inputs["x"], f32)
    W1 = np.asarray(inputs["W1"], f32)
    W2 = np.asarray(inputs["W2"], f32)
    W3 = np.asarray(inputs["W3"], f32)
    W4 = np.asarray(inputs["W4"], f32)
    b1 = np.asarray(inputs["b1"], f32)
    b2 = np.asarray(inputs["b2"], f32)
    b3 = np.asarray(inputs["b3"], f32)
    b4 = np.asarray(inputs["b4"], f32)

    def fold(g, be, m, v, b):
        s = (g / np.sqrt(v + EPS)).astype(f32)
        t = (b * s + be - m * s).astype(f32)
        return s, t

    s1, t1 = fold(np.asarray(inputs["g1"], f32), np.asarray(inputs["be1"], f32),
                  np.asarray(inputs["m1"], f32), np.asarray(inputs["v1"], f32), b1)
    s2, t2 = fold(np.asarray(inputs["g2"], f32), np.asarray(inputs["be2"], f32),
                  np.asarray(inputs["m2"], f32), np.asarray(inputs["v2"], f32), b2)
    s3, t3 = fold(np.asarray(inputs["g3"], f32), np.asarray(inputs["be3"], f32),
                  np.asarray(inputs["m3"], f32), np.asarray(inputs["v3"], f32), b3)

    def sgn(w):
        return np.where(w >= 0, f32(1.0), f32(-1.0))

    def dr_pack(wsT):
        # [768, 768] -> DoubleRow swizzle [128, 3*2*768] fp8:
        # out[p, t, j, m] = wsT[t*256 + j*128 + p, m]
        a = wsT.reshape(HP, 2, 128, H).transpose(2, 0, 1, 3)
        return np.ascontiguousarray(a).reshape(128, HP * 2 * H).astype(NP_F8)

    w1sT = sgn(W1).T                                   # [784, 768]
    w1big = np.ascontiguousarray(
        np.concatenate([w1sT, w1sT * f32(2.0 ** -12)], 0)
    ).astype(np.float16)                               # [1568, 768]
    w2big = dr_pack(sgn(W2).T)
    w3big = dr_pack(sgn(W3).T)
    w4big = np.ascontiguousarray(W4.T).astype(np.float16)   # [768, 10]

    par = np.ascontiguousarray(np.concatenate(
        [s1.reshape(6, 128).T, t1.reshape(6, 128).T,
         s2.reshape(6, 128).T, t2.reshape(6, 128).T,
         s3.reshape(6, 128).T, t3.reshape(6, 128).T], axis=1
    )).astype(f32)                                     # [128, 36]
    b4b = np.ascontiguousarray(np.broadcast_to(b4, (128, C))).astype(f32)

    xT = np.ascontiguousarray(x.T)                     # [784, 16384]
    xhi = xT.astype(np.float16)
    xlo = ((xT - xhi.astype(f32)) * f32(4096.0)).astype(np.float16)
    xbig = np.concatenate([xhi, xlo], 0)               # [1568, 16384]

    in_maps = []
    for i in range(NCORES):
        sl = np.ascontiguousarray(xbig[:, i * BC : (i + 1) * BC])
        in_maps.append({
            "xbig": sl, "w1": w1big, "w2": w2big, "w3": w3big,
            "w4": w4big, "par": par, "b4b": b4b,
        })
    return in_maps


TRACE = False
LAST_RESULT = None


def kernel(**inputs):
    global LAST_RESULT
    if "nc" not in _cache:
        _cache["nc"] = _build()
    nc = _cache["nc"]
    in_maps = _prep(inputs)
    res = run_bass_kernel_spmd(nc, in_maps, list(range(NCORES)), trace=TRACE)
    LAST_RESULT = res
    out = np.concatenate([np.asarray(r["out"]) for r in res.results], axis=0)
    return out.astype(np.float32)
